# revision 15
# baseline (speedup 1.0000x reference)
"""AttentionPool Trainium2 kernel: 8-core data-parallel Bass/Tile implementation.

Reference computation (per batch b of 32, S=2048, D=1024):
    xn = LayerNorm(x[b])                      # over D, eps 1e-5
    h = tanh(xn @ W1 + b1)
    scores = h @ W2 + b2                      # [S]
    w = softmax(scores)
    out[b] = sum_s w[s] * x[b, s, :]

Strategy: batch axis sharded over 8 cores (4 batches each). Per core/batch:
  - x loaded once with an f32->bf16 cast during the SWDGE DMA; the bf16
    natural-layout copy stays in SBUF and later feeds the pooling matmuls
    (no second HBM read of x).
  - LN stats via bn_stats/bn_aggr on the bf16 copy + Newton rsqrt (DVE),
    normalize with one fused tensor_scalar per subtile (bf16 fast path).
  - xn written to a DRAM scratch with a bf16->fp8e4 cast (SWDGE); read
    back via DMA-xbar transpose at 2-byte granularity, which yields fp8
    PAIRS per partition -- exactly the [K,2,N] moving-operand layout that
    MatmulPerfMode.DoubleRow contracts over (256-deep K per matmul).
  - matmul1 in fp8 DoubleRow (2x PE throughput), weights pre-packed on
    host in (super-tile, partition, plane) order and scaled by 32 so W1
    uses the e4m3 range; the 1/32 undo rides the tanh activation's scale.
  - tanh+c2 on ACT over paired [128,1024] PSUM tiles; scores matmul in
    bf16, all 4 chunks accumulated in ONE PSUM bank at partitions
    0/32/64/96 (tile_position col-tiling), emitted one e-group late so
    ACT latency never stalls the in-order PE queue.
  - b2 is dropped: softmax is invariant to a uniform score shift.
  - exp scores bounce through DRAM to repack [2048] -> [128,16]; Z comes
    from a [1,2048] re-read + DVE reduce (keeps everything partition-0).
  - pooling matmuls in bf16 against the SBUF-resident x copy, d-halves
    done sequentially through one PSUM row (bank budget: 6+1+1 = 8).
Batches are software-pipelined: phase1(b+1) (load/LN/quarter-transposes)
overlaps phase3(b) (matmuls); pooling of b-1 is sandwiched after the
first two e-groups of batch b so the scatter round-trip stays hidden.
Host-side prep folds ln_gamma into W1 and ln_beta@W1+b1 into c2.
"""
import sys
import os

sys.path.insert(0, '/opt/trn_rl_repo')

import numpy as np

import concourse.bass as bass
import concourse.tile as tile
from concourse import bacc, mybir
from concourse.bass_utils import run_bass_kernel_spmd

P = 128
D = 1024
S = 2048
B = 32
NCORES = 8
BLOC = B // NCORES            # batches per core
ROWS = BLOC * S               # 8192 rows per core
SUBT = S // P                 # 16 subtiles per batch
NG = 4                        # subtiles per stats/normalize group (= quarter)
CHUNK = 512                   # matmul moving free dim (output cols)
NCHUNK = S // CHUNK           # 4 chunks per batch
ET = D // P                   # 8 e-tiles

FP8 = True                    # matmul1 via fp8 DoubleRow
W1SCALE = 32.0                # host scales W1 by this; undone in tanh's scale
KT = 4 if FP8 else 8          # contraction super-tiles for matmul1
NPT = 4 if FP8 else 8         # transposed partition-tiles per batch

f32 = mybir.dt.float32
bf16 = mybir.dt.bfloat16
fp8 = mybir.dt.float8e4
AF = mybir.ActivationFunctionType
ALU = mybir.AluOpType
DR = mybir.MatmulPerfMode.DoubleRow


def build_nc():
    nc = bacc.Bacc("TRN2", target_bir_lowering=False, num_devices=NCORES)

    x = nc.dram_tensor("x", [ROWS, D], f32, kind="ExternalInput")
    if FP8:
        w1p = nc.dram_tensor("w1p", [P, KT, 2, ET, P], fp8,
                             kind="ExternalInput")
    else:
        w1p = nc.dram_tensor("w1p", [P, KT, ET, P], bf16,
                             kind="ExternalInput")
    c2v = nc.dram_tensor("c2v", [D], f32, kind="ExternalInput")
    w2v = nc.dram_tensor("w2v", [D], bf16, kind="ExternalInput")
    out = nc.dram_tensor("out", [BLOC, D], f32, kind="ExternalOutput")

    with tile.TileContext(nc) as tc:
        with (
            tc.tile_pool(name="consts", bufs=1) as consts,
            tc.tile_pool(name="xa", bufs=3) as xap,        # [128,16,1024] bf16
            tc.tile_pool(name="stats", bufs=8) as statp,
            tc.tile_pool(name="xnst", bufs=3) as xnst,     # [128,4,1024] bf16
            tc.tile_pool(name="xt", bufs=2) as xtp,        # [128,NPT,2048] bf16
            tc.tile_pool(name="ht", bufs=4) as htp,        # [128,1024] bf16
            tc.tile_pool(name="sc", bufs=4) as scp,        # small score tiles
            tc.tile_pool(name="zb", bufs=1) as zbp,        # [1,2048] Z re-read
            tc.tile_pool(name="ob", bufs=2) as obp,
            tc.tile_pool(name="psmm", bufs=3, space="PSUM") as psmm,  # 2 banks
            tc.tile_pool(name="pssc", bufs=1, space="PSUM") as pssc,  # 1 bank
            tc.tile_pool(name="pspl", bufs=1, space="PSUM") as pspl,  # 1 bank
            tc.tile_pool(name="dram", bufs=8, space="DRAM") as dramp,
        ):
            # ---- constants ----
            w1_sb = consts.tile(list(w1p.shape), fp8 if FP8 else bf16)
            nc.scalar.dma_start(w1_sb, w1p.ap())
            c2_sb = consts.tile([P, ET], f32)
            nc.scalar.dma_start(c2_sb, c2v.ap().rearrange("(t p) -> p t", p=P))
            w2_sb = consts.tile([P, ET], bf16)
            nc.scalar.dma_start(w2_sb, w2v.ap().rearrange("(t p) -> p t", p=P))
            ones_sb = consts.tile([P, 1], bf16)
            nc.vector.memset(ones_sb, 1.0)
            x3 = x.ap().rearrange("(b t p) d -> b t p d", b=BLOC, p=P)

            def phase1(b, xa, xtt):
                """Load+cast x, LN stats, normalize -> fp8 scratch, quarter
                transposes as soon as each quarter's rows are staged.

                Each quarter gets its own DRAM scratch tile so a quarter's
                transposes depend only on that quarter's write (region
                tracking through the bitcast view is tile-granular)."""
                for g in range(NG):
                    t0 = NG * g
                    if b == 0:
                        for s2 in range(0, NG, 2):
                            nc.gpsimd.dma_start(
                                xa[:, t0 + s2:t0 + s2 + 2, :],
                                x3[b, t0 + s2:t0 + s2 + 2].rearrange(
                                    "t p d -> p t d"))
                    else:
                        nc.gpsimd.dma_start(
                            xa[:, t0:t0 + NG, :],
                            x3[b, t0:t0 + NG].rearrange("t p d -> p t d"))
                    mv = statp.tile([P, NG, 2], f32, tag="mv")
                    for s in range(NG):
                        st = statp.tile([P, 2, 6], f32, tag="bnst")
                        nc.vector.bn_stats(st[:, 0, :], xa[:, t0 + s, 0:512])
                        nc.vector.bn_stats(st[:, 1, :], xa[:, t0 + s, 512:1024])
                        nc.vector.bn_aggr(mv[:, s, :], st)
                    # rstd = rsqrt(var+eps): quake seed + 2 Newton steps (DVE)
                    var = statp.tile([P, NG], f32, tag="var")
                    nc.vector.tensor_scalar(out=var, in0=mv[:, :, 1],
                                            scalar1=1e-5, scalar2=0.5,
                                            op0=ALU.add, op1=ALU.mult)
                    y = statp.tile([P, NG], f32, tag="y")
                    yi = y.bitcast(mybir.dt.int32)
                    vi = var.bitcast(mybir.dt.int32)
                    nc.vector.tensor_scalar(out=yi, in0=vi, scalar1=0x800000,
                                            scalar2=None, op0=ALU.add)
                    nc.vector.tensor_scalar(out=yi, in0=yi, scalar1=1,
                                            scalar2=None,
                                            op0=ALU.logical_shift_right)
                    nc.vector.tensor_scalar(out=yi, in0=yi, scalar1=-1,
                                            scalar2=0x5f3759df,
                                            op0=ALU.mult, op1=ALU.add)
                    tny = statp.tile([P, NG], f32, tag="tny")
                    for _ in range(2):
                        nc.vector.tensor_tensor(tny, y, y, ALU.mult)
                        nc.vector.tensor_tensor(tny, tny, var, ALU.mult)
                        nc.vector.tensor_scalar(out=tny, in0=tny, scalar1=-1.0,
                                                scalar2=1.5,
                                                op0=ALU.mult, op1=ALU.add)
                        nc.vector.tensor_tensor(y, y, tny, ALU.mult)
                    xnb = xnst.tile([P, NG, D], bf16, tag="xnst")
                    for s in range(NG):
                        nc.vector.tensor_scalar(out=xnb[:, s, :],
                                                in0=xa[:, t0 + s, :],
                                                scalar1=mv[:, s, 0:1],
                                                scalar2=y[:, s:s + 1],
                                                op0=ALU.subtract, op1=ALU.mult)
                    scr_q = dramp.tile([CHUNK, D], fp8 if FP8 else bf16,
                                       tag="scratch")
                    scrT = scr_q.bitcast(bf16)        # [512, D//2] pair view
                    nc.gpsimd.dma_start(
                        scr_q.rearrange("(t p) d -> t p d", p=P).rearrange(
                            "t p d -> p t d"), xnb)
                    # quarter g rows are now staged: transpose them
                    # (sync only: concurrent transposes on both HWDGE rings
                    # produced corrupted reads on hardware)
                    for t in range(NPT):
                        nc.sync.dma_start_transpose(
                            xtt[:, t, g * CHUNK:(g + 1) * CHUNK],
                            scrT[:, t * P:(t + 1) * P])

            def emit_pool_mms(pl_ps, epk, xa, c=None):
                """Pooling matmuls, subtiles of chunk c (or all 16).
                Both d-halves per subtile back-to-back (shared stationary
                epk column -> LDWEIGHTS dedup): half 0 -> row 0, half 1 ->
                row 32 of the same PSUM bank."""
                rng = range(4 * c, 4 * c + 4) if c is not None else range(SUBT)
                for t in rng:
                    s = (t - 4 * c) if c is not None else t
                    nc.tensor.matmul(pl_ps[0:1, :], epk[:, s:s + 1],
                                     xa[:, t, 0:512],
                                     start=(t == 0), stop=(t == SUBT - 1),
                                     tile_position=(0, 0))
                    nc.tensor.matmul(pl_ps[32:33, :], epk[:, s:s + 1],
                                     xa[:, t, 512:1024],
                                     start=(t == 0), stop=(t == SUBT - 1),
                                     tile_position=(0, 32))

            def z_chain(eb):
                """Z = sum(exp scores) via [1,2048] re-reads + DVE reduce.
                Duplicated at partitions 0 and 32 so both pooling rows have
                a partition-aligned 1/Z for their ACT copy."""
                ztv = zbp.tile([33, S], f32, tag="ztv")
                zt = scp.tile([33, 1], f32, tag="zt")
                rz = scp.tile([33, 1], f32, tag="rz")
                for r in (0, 32):
                    nc.scalar.dma_start(ztv[r:r + 1, :],
                                        eb.rearrange("(a s) -> a s", a=1))
                    nc.vector.tensor_reduce(zt[r:r + 1, :], ztv[r:r + 1, :],
                                            axis=mybir.AxisListType.X,
                                            op=ALU.add)
                    nc.vector.reciprocal(rz[r:r + 1, :], zt[r:r + 1, :])
                return rz

            def pool_store(b, pl_ps, rz):
                """Scaled copies from PSUM rows 0/32 + the two out stores.

                The half-1 copy reads partition 32 and must write an SBUF
                tile at partition 32 (engines cannot shift partitions);
                rz0/rz32 are per-row copies of 1/Z for the same reason."""
                ob0 = obp.tile([1, 512], f32, tag="ob0")
                nc.scalar.activation(ob0, pl_ps[0:1, :], AF.Copy,
                                     scale=rz[0:1, 0:1])
                nc.sync.dma_start(out.ap()[b:b + 1, 0:512], ob0)
                ob1 = obp.tile([33, 512], f32, tag="ob1")
                nc.scalar.activation(ob1[32:33, :], pl_ps[32:33, :], AF.Copy,
                                     scale=rz[32:33, 0:1])
                nc.sync.dma_start(out.ap()[b:b + 1, 512:1024], ob1[32:33, :])

            def phase4(b, epk_f, eb, xa):
                """Batch-level pooling for a non-last batch."""
                rz = z_chain(eb)
                epk = scp.tile([P, SUBT], bf16, tag="epk")
                nc.vector.tensor_copy(epk, epk_f)
                pl_ps = pspl.tile([P, 512], f32, tag="pspl")
                emit_pool_mms(pl_ps, epk, xa)
                pool_store(b, pl_ps, rz)

            def phase3_pass(b, xa, xtt, pairs, sc_ps, prev):
                """matmul1 + tanh + scores for the chunk-pairs in `pairs`."""
                f8 = xtt.bitcast(fp8) if FP8 else None   # [128,KT,4096]
                hts = [None] * ET

                def rhs(t, c):
                    if FP8:
                        return f8[:, t, c * 2 * CHUNK:(c + 1) * 2 * CHUNK] \
                            .rearrange("p (s two) -> p two s", two=2)
                    return xtt[:, t, c * CHUNK:(c + 1) * CHUNK]

                def lhs(t, e):
                    if FP8:
                        return w1_sb[:, t, :, e, :]
                    return w1_sb[:, t, e, :]

                def emit_sc(e):
                    for hti, (ca, cb) in zip(hts[e], pairs):
                        for c in (ca, cb):
                            nc.tensor.matmul(
                                sc_ps[32 * c:32 * c + 1, :], w2_sb[:, e:e + 1],
                                hti[:, (c % 2) * CHUNK:(c % 2 + 1) * CHUNK],
                                start=(e == 0), stop=(e == ET - 1),
                                tile_position=(0, 32 * c))

                pm = DR if FP8 else None
                tanh_scale = (1.0 / W1SCALE) if FP8 else 1.0
                for e in range(ET):
                    etiles = []
                    for (ca, cb) in pairs:
                        ps = psmm.tile([P, 2 * CHUNK], f32, tag="mm")
                        for t in range(KT):
                            nc.tensor.matmul(ps[:, 0:CHUNK], lhs(t, e),
                                             rhs(t, ca),
                                             start=(t == 0), stop=(t == KT - 1),
                                             perf_mode=pm)
                            nc.tensor.matmul(ps[:, CHUNK:2 * CHUNK], lhs(t, e),
                                             rhs(t, cb),
                                             start=(t == 0), stop=(t == KT - 1),
                                             perf_mode=pm)
                        hti = htp.tile([P, 2 * CHUNK], bf16, tag="ht")
                        nc.scalar.activation(hti, ps, AF.Tanh,
                                             bias=c2_sb[:, e:e + 1],
                                             scale=tanh_scale)
                        etiles.append(hti)
                    hts[e] = etiles
                    if e >= 1:
                        emit_sc(e - 1)
                    if e == 1 and prev is not None:
                        phase4(*prev)
                        prev = None
                emit_sc(ET - 1)
                return prev

            def phase3(b, xa, xtt, prev):
                """Full phase3: matmul passes, exp + scatter (+ inline
                pooling for the last batch)."""
                last = (b == BLOC - 1)
                sc_ps = pssc.tile([P, CHUNK], f32, tag="pssc")
                if b == 0:
                    # quarters stream in: run chunk-pairs as separate passes
                    phase3_pass(b, xa, xtt, [(0, 1)], sc_ps, None)
                    phase3_pass(b, xa, xtt, [(2, 3)], sc_ps, None)
                else:
                    prev = phase3_pass(b, xa, xtt, [(0, 1), (2, 3)], sc_ps,
                                       prev)
                    assert prev is None

                ec = scp.tile([P, CHUNK], f32, tag="ec")
                eb = dramp.tile([S], f32, tag="eb")
                if not last:
                    for c in range(NCHUNK):
                        nc.scalar.activation(ec[32 * c:32 * c + 1, :],
                                             sc_ps[32 * c:32 * c + 1, :],
                                             AF.Exp)
                    nc.scalar.dma_start(
                        eb.rearrange("(c j) -> c j", c=NCHUNK),
                        ec.rearrange("(a b) f -> a b f", b=32)[:, 0, :])
                    epk_f = scp.tile([P, SUBT], f32, tag="epkf")
                    nc.scalar.dma_start(
                        epk_f, eb.rearrange("(t p) -> p t", p=P))
                    return (b, epk_f, eb, xa)

                # last batch: per-chunk scatter + inline pooling
                pl_ps = pspl.tile([P, 512], f32, tag="pspl")
                for c in range(NCHUNK):
                    nc.scalar.activation(ec[32 * c:32 * c + 1, :],
                                         sc_ps[32 * c:32 * c + 1, :],
                                         AF.Exp)
                    nc.scalar.dma_start(eb[c * CHUNK:(c + 1) * CHUNK],
                                        ec[32 * c:32 * c + 1, :])
                    epk_f = scp.tile([P, NCHUNK], f32, tag="epkf")
                    nc.scalar.dma_start(
                        epk_f,
                        eb[c * CHUNK:(c + 1) * CHUNK].rearrange(
                            "(t p) -> p t", p=P))
                    epk = scp.tile([P, NCHUNK], bf16, tag="epk")
                    nc.vector.tensor_copy(epk, epk_f)
                    emit_pool_mms(pl_ps, epk, xa, c=c)
                rz = z_chain(eb)
                pool_store(b, pl_ps, rz)
                return None

            prev = None
            for b in range(BLOC):
                xa = xap.tile([P, SUBT, D], bf16, tag="xa", name=f"xa{b}")
                xtt = xtp.tile([P, NPT, S], bf16, tag="xt", name=f"xt{b}")
                phase1(b, xa, xtt)
                prev = phase3(b, xa, xtt, prev)
            assert prev is None

    nc.compile()
    return nc


_NC_CACHE = {}


def _get_nc():
    if "nc" not in _NC_CACHE:
        _NC_CACHE["nc"] = build_nc()
    return _NC_CACHE["nc"]


def _prep_host(ln_gamma, ln_beta, W1, b1, W2, b2):
    import ml_dtypes
    W1g = (np.asarray(ln_gamma, np.float32)[:, None]
           * np.asarray(W1, np.float32))
    if FP8:
        # pack rows in DoubleRow (super-tile, partition, plane) order:
        # d = t*256 + p*2 + i  ->  arr[p, t, i, e8, e128]
        W1s = (W1g * W1SCALE).astype(ml_dtypes.float8_e4m3)
        W1pk = np.ascontiguousarray(
            W1s.reshape(KT, P, 2, ET, P).transpose(1, 0, 2, 3, 4))
    else:
        # d = t*128 + p  ->  arr[p, t, e8, e128]
        W1s = W1g.astype(ml_dtypes.bfloat16)
        W1pk = np.ascontiguousarray(
            W1s.reshape(KT, P, ET, P).transpose(1, 0, 2, 3))
    c2 = (np.asarray(ln_beta, np.float32) @ np.asarray(W1, np.float32)
          + np.asarray(b1, np.float32))
    w2v = np.ascontiguousarray(
        np.asarray(W2, np.float32)[:, 0]).astype(ml_dtypes.bfloat16)
    return W1pk, np.ascontiguousarray(c2), w2v


def run_cores(inputs, trace=False, **kw):
    x = np.asarray(inputs["x"], np.float32)
    W1pk, c2, w2v = _prep_host(inputs["ln_gamma"], inputs["ln_beta"],
                               inputs["W1"], inputs["b1"],
                               inputs["W2"], inputs["b2"])
    nc = _get_nc()
    in_maps = []
    for c in range(NCORES):
        shard = np.ascontiguousarray(
            x[c * BLOC:(c + 1) * BLOC].reshape(ROWS, D))
        in_maps.append(dict(x=shard, w1p=W1pk, c2v=c2, w2v=w2v))
    res = run_bass_kernel_spmd(nc, in_maps, core_ids=list(range(NCORES)),
                               trace=trace, **kw)
    full = np.concatenate([res.results[c]["out"] for c in range(NCORES)],
                          axis=0)
    return full, res


def kernel(**inputs) -> np.ndarray:
    out, _ = run_cores(inputs, trace=False)
    return out.astype(np.float32)


# revision 17
# speedup vs baseline: 1.2474x; 1.2474x over previous
"""AttentionPool Trainium2 kernel: 8-core data-parallel Bass/Tile implementation.

Reference computation (per batch b of 32, S=2048, D=1024):
    xn = LayerNorm(x[b])                      # over D, eps 1e-5
    h = tanh(xn @ W1 + b1)
    scores = h @ W2 + b2                      # [S]
    w = softmax(scores)
    out[b] = sum_s w[s] * x[b, s, :]

Strategy: batch axis sharded over 8 cores (4 batches each). Per core/batch:
  - x loaded once with an f32->bf16 cast during the SWDGE DMA; the bf16
    natural-layout copy stays in SBUF and later feeds the pooling matmuls
    (no second HBM read of x).
  - LN stats via bn_stats/bn_aggr on the bf16 copy + Newton rsqrt (DVE);
    normalize emits fp8e4 directly, split between ACT (scale/bias per
    partition) and DVE (tensor_scalar) to balance the two engines.
  - fp8 xn staged to a per-quarter DRAM scratch via plain sync-HWDGE DMA
    (keeps the critical write off the SWDGE ring that the big loads use),
    then read back via DMA-xbar transpose at 2-byte granularity: fp8 PAIRS
    per partition = exactly the [K,2,N] moving layout DoubleRow contracts.
  - matmul1 in fp8 DoubleRow (half the K-tiles of bf16), weights packed on
    host in (super-tile, partition, plane) order, scaled by 32 for e4m3;
    the 1/32 undo rides the tanh activation's scale.
  - tanh+c2 on ACT over [128, G*512] PSUM tiles; scores matmul in bf16,
    all 4 chunks accumulated in ONE PSUM bank at partitions 0/32/64/96
    (tile_position col-tiling), emitted one e-group late so ACT latency
    never stalls the in-order PE queue.
  - b2 dropped (softmax shift-invariance); Z comes from the exp ops' ACT
    accumulator (4 per-chunk partials at partitions 0/32/64/96), bounced
    through DRAM to partitions 0 and 32 for the final 1/Z scaling.
  - pooling matmuls in bf16 against the SBUF-resident x copy, both
    d-halves per subtile back-to-back into rows 0/32 of one PSUM bank
    (shared stationary -> LDWEIGHTS dedup).
Batches are software-pipelined: phase1(b+1) (load/LN/quarter-transposes)
overlaps phase3(b); pooling of b-1 is sandwiched after the first two
e-groups of batch b. Batch 0 runs 4 single-chunk passes so the PE can
start as soon as the first quarter is transposed.
Host-side prep folds ln_gamma into W1 and ln_beta@W1+b1 into c2.
"""
import sys
import os

sys.path.insert(0, '/opt/trn_rl_repo')

import numpy as np

import concourse.bass as bass
import concourse.tile as tile
from concourse import bacc, mybir
from concourse.bass_utils import run_bass_kernel_spmd

P = 128
D = 1024
S = 2048
B = 32
NCORES = 8
BLOC = B // NCORES            # batches per core
ROWS = BLOC * S               # 8192 rows per core
SUBT = S // P                 # 16 subtiles per batch
NG = 4                        # subtiles per stats/normalize group (= quarter)
CHUNK = 512                   # matmul moving free dim (output cols)
NCHUNK = S // CHUNK           # 4 chunks per batch
ET = D // P                   # 8 e-tiles

FP8 = True                    # matmul1 via fp8 DoubleRow
W1SCALE = 32.0                # host scales W1 by this; undone in tanh's scale
KT = 4 if FP8 else 8          # contraction super-tiles for matmul1
NPT = 4 if FP8 else 8         # transposed partition-tiles per batch

f32 = mybir.dt.float32
bf16 = mybir.dt.bfloat16
fp8 = mybir.dt.float8e4
AF = mybir.ActivationFunctionType
ALU = mybir.AluOpType
DR = mybir.MatmulPerfMode.DoubleRow
XDT = fp8 if FP8 else bf16    # staged-xn dtype


def build_nc():
    nc = bacc.Bacc("TRN2", target_bir_lowering=False, num_devices=NCORES)

    x = nc.dram_tensor("x", [ROWS, D], f32, kind="ExternalInput")
    if FP8:
        w1p = nc.dram_tensor("w1p", [P, KT, 2, ET, P], fp8,
                             kind="ExternalInput")
    else:
        w1p = nc.dram_tensor("w1p", [P, KT, ET, P], bf16,
                             kind="ExternalInput")
    c2v = nc.dram_tensor("c2v", [D], f32, kind="ExternalInput")
    w2v = nc.dram_tensor("w2v", [D], bf16, kind="ExternalInput")
    out = nc.dram_tensor("out", [BLOC, D], f32, kind="ExternalOutput")

    with tile.TileContext(nc) as tc:
        with (
            tc.tile_pool(name="consts", bufs=1) as consts,
            tc.tile_pool(name="xa", bufs=3) as xap,        # [128,16,1024] bf16
            tc.tile_pool(name="stats", bufs=8) as statp,
            tc.tile_pool(name="xnst", bufs=3) as xnst,     # [128,4,1024] fp8
            tc.tile_pool(name="xt", bufs=2) as xtp,        # [128,NPT,2048] bf16
            tc.tile_pool(name="ht", bufs=4) as htp,        # [128,<=1024] bf16
            tc.tile_pool(name="sc", bufs=5) as scp,        # small score tiles
            tc.tile_pool(name="ob", bufs=2) as obp,
            tc.tile_pool(name="psmm", bufs=3, space="PSUM") as psmm,  # 2 banks
            tc.tile_pool(name="pssc", bufs=1, space="PSUM") as pssc,  # 1 bank
            tc.tile_pool(name="pspl", bufs=1, space="PSUM") as pspl,  # 1 bank
            tc.tile_pool(name="dram", bufs=8, space="DRAM") as dramp,
        ):
            # ---- constants ----
            w1_sb = consts.tile(list(w1p.shape), fp8 if FP8 else bf16)
            nc.scalar.dma_start(w1_sb, w1p.ap())
            c2_sb = consts.tile([P, ET], f32)
            nc.scalar.dma_start(c2_sb, c2v.ap().rearrange("(t p) -> p t", p=P))
            w2_sb = consts.tile([P, ET], bf16)
            nc.scalar.dma_start(w2_sb, w2v.ap().rearrange("(t p) -> p t", p=P))
            x3 = x.ap().rearrange("(b t p) d -> b t p d", b=BLOC, p=P)

            def phase1(b, xa, xtt):
                """Load+cast x, LN stats, normalize -> fp8 scratch quarter,
                transpose each quarter as soon as its rows are staged."""
                for g in range(NG):
                    t0 = NG * g
                    if b == 0:
                        for s2 in range(0, NG, 2):
                            nc.gpsimd.dma_start(
                                xa[:, t0 + s2:t0 + s2 + 2, :],
                                x3[b, t0 + s2:t0 + s2 + 2].rearrange(
                                    "t p d -> p t d"))
                    else:
                        nc.gpsimd.dma_start(
                            xa[:, t0:t0 + NG, :],
                            x3[b, t0:t0 + NG].rearrange("t p d -> p t d"))
                    mv = statp.tile([P, NG, 2], f32, tag="mv")
                    for s in range(NG):
                        st = statp.tile([P, 2, 6], f32, tag="bnst")
                        nc.vector.bn_stats(st[:, 0, :], xa[:, t0 + s, 0:512])
                        nc.vector.bn_stats(st[:, 1, :], xa[:, t0 + s, 512:1024])
                        nc.vector.bn_aggr(mv[:, s, :], st)
                    # rstd = rsqrt(var+eps): quake seed + 2 Newton steps (DVE)
                    var = statp.tile([P, NG], f32, tag="var")
                    nc.vector.tensor_scalar(out=var, in0=mv[:, :, 1],
                                            scalar1=1e-5, scalar2=0.5,
                                            op0=ALU.add, op1=ALU.mult)
                    y = statp.tile([P, NG], f32, tag="y")
                    yi = y.bitcast(mybir.dt.int32)
                    vi = var.bitcast(mybir.dt.int32)
                    nc.vector.tensor_scalar(out=yi, in0=vi, scalar1=0x800000,
                                            scalar2=None, op0=ALU.add)
                    nc.vector.tensor_scalar(out=yi, in0=yi, scalar1=1,
                                            scalar2=None,
                                            op0=ALU.logical_shift_right)
                    nc.vector.tensor_scalar(out=yi, in0=yi, scalar1=-1,
                                            scalar2=0x5f3759df,
                                            op0=ALU.mult, op1=ALU.add)
                    tny = statp.tile([P, NG], f32, tag="tny")
                    for _ in range(2):
                        nc.vector.tensor_tensor(tny, y, y, ALU.mult)
                        nc.vector.tensor_tensor(tny, tny, var, ALU.mult)
                        nc.vector.tensor_scalar(out=tny, in0=tny, scalar1=-1.0,
                                                scalar2=1.5,
                                                op0=ALU.mult, op1=ALU.add)
                        nc.vector.tensor_tensor(y, y, tny, ALU.mult)
                    # mb = -mu * rstd (ACT normalize bias)
                    mb = statp.tile([P, NG], f32, tag="mb")
                    nc.vector.tensor_tensor(mb, mv[:, :, 0], y, ALU.mult)
                    nc.vector.tensor_scalar(out=mb, in0=mb, scalar1=-1.0,
                                            scalar2=None, op0=ALU.mult)
                    xnb = xnst.tile([P, NG, D], XDT, tag="xnst")
                    for s in range(NG):
                        if s % 2 == 0:
                            nc.scalar.activation(xnb[:, s, :],
                                                 xa[:, t0 + s, :], AF.Identity,
                                                 scale=y[:, s:s + 1],
                                                 bias=mb[:, s:s + 1])
                        else:
                            nc.vector.tensor_scalar(out=xnb[:, s, :],
                                                    in0=xa[:, t0 + s, :],
                                                    scalar1=mv[:, s, 0:1],
                                                    scalar2=y[:, s:s + 1],
                                                    op0=ALU.subtract,
                                                    op1=ALU.mult)
                    scr_q = dramp.tile([CHUNK, D], XDT, tag="scratch")
                    scrT = scr_q.bitcast(bf16)        # [512, D//2] pair view
                    nc.sync.dma_start(
                        scr_q.rearrange("(t p) d -> t p d", p=P).rearrange(
                            "t p d -> p t d"), xnb)
                    # quarter g rows staged: transpose them (sync HWDGE only;
                    # concurrent transposes on both rings corrupt on HW)
                    for t in range(NPT):
                        nc.sync.dma_start_transpose(
                            xtt[:, t, g * CHUNK:(g + 1) * CHUNK],
                            scrT[:, t * P:(t + 1) * P])

            def emit_pool_mms(pl_ps, epk, xa, c=None):
                """Pooling matmuls, subtiles of chunk c (or all 16).
                Both d-halves per subtile back-to-back (shared stationary
                epk column -> LDWEIGHTS dedup): half 0 -> row 0, half 1 ->
                row 32 of the same PSUM bank."""
                rng = range(4 * c, 4 * c + 4) if c is not None else range(SUBT)
                for t in rng:
                    s = (t - 4 * c) if c is not None else t
                    nc.tensor.matmul(pl_ps[0:1, :], epk[:, s:s + 1],
                                     xa[:, t, 0:512],
                                     start=(t == 0), stop=(t == SUBT - 1),
                                     tile_position=(0, 0))
                    nc.tensor.matmul(pl_ps[32:33, :], epk[:, s:s + 1],
                                     xa[:, t, 512:1024],
                                     start=(t == 0), stop=(t == SUBT - 1),
                                     tile_position=(0, 32))

            def z_chain(zc, zb):
                """1/Z at partitions 0 and 32 from the 4 per-chunk exp
                accumulator partials (at partitions 0/32/64/96) via a tiny
                DRAM bounce."""
                nc.scalar.dma_start(
                    zb, zc.rearrange("(a b) f -> a b f", b=32)[:, 0, :])
                z4 = scp.tile([33, NCHUNK], f32, tag="z4")
                zt = scp.tile([33, 1], f32, tag="zt")
                rz = scp.tile([33, 1], f32, tag="rz")
                for r in (0, 32):
                    nc.scalar.dma_start(z4[r:r + 1, :],
                                        zb.rearrange("(a c) -> a c", a=1))
                    nc.vector.tensor_reduce(zt[r:r + 1, :], z4[r:r + 1, :],
                                            axis=mybir.AxisListType.X,
                                            op=ALU.add)
                    nc.vector.reciprocal(rz[r:r + 1, :], zt[r:r + 1, :])
                return rz

            def pool_store(b, pl_ps, rz):
                """Scaled copies from PSUM rows 0/32 + the two out stores."""
                ob0 = obp.tile([1, 512], f32, tag="ob0")
                nc.scalar.activation(ob0, pl_ps[0:1, :], AF.Copy,
                                     scale=rz[0:1, 0:1])
                nc.sync.dma_start(out.ap()[b:b + 1, 0:512], ob0)
                ob1 = obp.tile([33, 512], f32, tag="ob1")
                nc.scalar.activation(ob1[32:33, :], pl_ps[32:33, :], AF.Copy,
                                     scale=rz[32:33, 0:1])
                nc.sync.dma_start(out.ap()[b:b + 1, 512:1024], ob1[32:33, :])

            def phase4(b, epk_f, zc, zb, xa):
                """Batch-level pooling for a non-last batch."""
                rz = z_chain(zc, zb)
                epk = scp.tile([P, SUBT], bf16, tag="epk")
                nc.vector.tensor_copy(epk, epk_f)
                pl_ps = pspl.tile([P, 512], f32, tag="pspl")
                emit_pool_mms(pl_ps, epk, xa)
                pool_store(b, pl_ps, rz)

            def phase3_pass(b, xa, xtt, groups, sc_ps, prev):
                """matmul1 + tanh + scores for the chunk-groups in `groups`
                (each group is a tuple of chunks sharing one PSUM tile)."""
                f8 = xtt.bitcast(fp8) if FP8 else None   # [128,KT,4096]
                hts = [None] * ET

                def rhs(t, c):
                    if FP8:
                        return f8[:, t, c * 2 * CHUNK:(c + 1) * 2 * CHUNK] \
                            .rearrange("p (s two) -> p two s", two=2)
                    return xtt[:, t, c * CHUNK:(c + 1) * CHUNK]

                def lhs(t, e):
                    if FP8:
                        return w1_sb[:, t, :, e, :]
                    return w1_sb[:, t, e, :]

                def emit_sc(e):
                    for hti, grp in zip(hts[e], groups):
                        for j, c in enumerate(grp):
                            nc.tensor.matmul(
                                sc_ps[32 * c:32 * c + 1, :], w2_sb[:, e:e + 1],
                                hti[:, j * CHUNK:(j + 1) * CHUNK],
                                start=(e == 0), stop=(e == ET - 1),
                                tile_position=(0, 32 * c))

                pm = DR if FP8 else None
                tanh_scale = (1.0 / W1SCALE) if FP8 else 1.0
                for e in range(ET):
                    etiles = []
                    for grp in groups:
                        w = len(grp) * CHUNK
                        ps = psmm.tile([P, w], f32, tag="mm")
                        for t in range(KT):
                            for j, c in enumerate(grp):
                                nc.tensor.matmul(
                                    ps[:, j * CHUNK:(j + 1) * CHUNK],
                                    lhs(t, e), rhs(t, c),
                                    start=(t == 0), stop=(t == KT - 1),
                                    perf_mode=pm)
                        hti = htp.tile([P, w], bf16, tag="ht")
                        nc.scalar.activation(hti, ps, AF.Tanh,
                                             bias=c2_sb[:, e:e + 1],
                                             scale=tanh_scale)
                        etiles.append(hti)
                    hts[e] = etiles
                    if e >= 1:
                        emit_sc(e - 1)
                    if e == 1 and prev is not None:
                        phase4(*prev)
                        prev = None
                emit_sc(ET - 1)
                return prev

            def phase3(b, xa, xtt, prev):
                """Full phase3: matmul passes, exp + scatter (+ inline
                pooling for the last batch)."""
                last = (b == BLOC - 1)
                sc_ps = pssc.tile([P, CHUNK], f32, tag="pssc")
                if b == 0:
                    # quarters stream in: single-chunk passes for fast ramp
                    for c in range(NCHUNK):
                        phase3_pass(b, xa, xtt, [(c,)], sc_ps, None)
                else:
                    prev = phase3_pass(b, xa, xtt, [(0, 1), (2, 3)], sc_ps,
                                       prev)
                    assert prev is None

                ec = scp.tile([P, CHUNK], f32, tag="ec")
                zc = scp.tile([P, 1], f32, tag="zc")
                eb = dramp.tile([S], f32, tag="eb")
                zb = dramp.tile([NCHUNK], f32, tag="zb")
                if not last:
                    for c in range(NCHUNK):
                        nc.scalar.activation(ec[32 * c:32 * c + 1, :],
                                             sc_ps[32 * c:32 * c + 1, :],
                                             AF.Exp,
                                             accum_out=zc[32 * c:32 * c + 1, :])
                    nc.scalar.dma_start(
                        eb.rearrange("(c j) -> c j", c=NCHUNK),
                        ec.rearrange("(a b) f -> a b f", b=32)[:, 0, :])
                    epk_f = scp.tile([P, SUBT], f32, tag="epkf")
                    nc.scalar.dma_start(
                        epk_f, eb.rearrange("(t p) -> p t", p=P))
                    return (b, epk_f, zc, zb, xa)

                # last batch: per-chunk scatter + inline pooling
                pl_ps = pspl.tile([P, 512], f32, tag="pspl")
                for c in range(NCHUNK):
                    nc.scalar.activation(ec[32 * c:32 * c + 1, :],
                                         sc_ps[32 * c:32 * c + 1, :],
                                         AF.Exp,
                                         accum_out=zc[32 * c:32 * c + 1, :])
                    nc.scalar.dma_start(eb[c * CHUNK:(c + 1) * CHUNK],
                                        ec[32 * c:32 * c + 1, :])
                    epk_f = scp.tile([P, NCHUNK], f32, tag="epkf")
                    nc.scalar.dma_start(
                        epk_f,
                        eb[c * CHUNK:(c + 1) * CHUNK].rearrange(
                            "(t p) -> p t", p=P))
                    epk = scp.tile([P, NCHUNK], bf16, tag="epk")
                    nc.vector.tensor_copy(epk, epk_f)
                    emit_pool_mms(pl_ps, epk, xa, c=c)
                rz = z_chain(zc, zb)
                pool_store(b, pl_ps, rz)
                return None

            prev = None
            for b in range(BLOC):
                xa = xap.tile([P, SUBT, D], bf16, tag="xa", name=f"xa{b}")
                xtt = xtp.tile([P, NPT, S], bf16, tag="xt", name=f"xt{b}")
                phase1(b, xa, xtt)
                prev = phase3(b, xa, xtt, prev)
            assert prev is None

    nc.compile()
    return nc


_NC_CACHE = {}


def _get_nc():
    if "nc" not in _NC_CACHE:
        _NC_CACHE["nc"] = build_nc()
    return _NC_CACHE["nc"]


def _prep_host(ln_gamma, ln_beta, W1, b1, W2, b2):
    import ml_dtypes
    W1g = (np.asarray(ln_gamma, np.float32)[:, None]
           * np.asarray(W1, np.float32))
    if FP8:
        # pack rows in DoubleRow (super-tile, partition, plane) order:
        # d = t*256 + p*2 + i  ->  arr[p, t, i, e8, e128]
        W1s = (W1g * W1SCALE).astype(ml_dtypes.float8_e4m3)
        W1pk = np.ascontiguousarray(
            W1s.reshape(KT, P, 2, ET, P).transpose(1, 0, 2, 3, 4))
    else:
        # d = t*128 + p  ->  arr[p, t, e8, e128]
        W1s = W1g.astype(ml_dtypes.bfloat16)
        W1pk = np.ascontiguousarray(
            W1s.reshape(KT, P, ET, P).transpose(1, 0, 2, 3))
    c2 = (np.asarray(ln_beta, np.float32) @ np.asarray(W1, np.float32)
          + np.asarray(b1, np.float32))
    w2v = np.ascontiguousarray(
        np.asarray(W2, np.float32)[:, 0]).astype(ml_dtypes.bfloat16)
    return W1pk, np.ascontiguousarray(c2), w2v


def run_cores(inputs, trace=False, **kw):
    x = np.asarray(inputs["x"], np.float32)
    W1pk, c2, w2v = _prep_host(inputs["ln_gamma"], inputs["ln_beta"],
                               inputs["W1"], inputs["b1"],
                               inputs["W2"], inputs["b2"])
    nc = _get_nc()
    in_maps = []
    for c in range(NCORES):
        shard = np.ascontiguousarray(
            x[c * BLOC:(c + 1) * BLOC].reshape(ROWS, D))
        in_maps.append(dict(x=shard, w1p=W1pk, c2v=c2, w2v=w2v))
    res = run_bass_kernel_spmd(nc, in_maps, core_ids=list(range(NCORES)),
                               trace=trace, **kw)
    full = np.concatenate([res.results[c]["out"] for c in range(NCORES)],
                          axis=0)
    return full, res


def kernel(**inputs) -> np.ndarray:
    out, _ = run_cores(inputs, trace=False)
    return out.astype(np.float32)
